# revision 15
# baseline (speedup 1.0000x reference)
"""Trainium2 Bass kernel for conditional MAF (MADE) forward pass.

Reference computation (N=65536, D=32, H=1024, C=64):
    Wc_h = context @ Wc
    repeat D times:  hid = tanh(y @ (W1*mask1) + Wc_h + b1)
                     params = hid @ (W2*mask2) + b2        # [N, D, 2]
                     y = x * exp(params[...,1]) + params[...,0]
    returns (y, params[...,1].sum(-1))

Strategy: pure data-parallel over 8 NeuronCores (8192 rows each).
On-chip layout keeps the batch on the free dimension (activations
transposed) so both matmul stages contract over partitions:
  stage 1: psum[128u, B] = Wa[:, chunk].T @ A[97, B]
           where A rows = [y (32) | context (64) | ones (1)] and
           Wa = [W1*mask1 ; Wc ; b1] -- context conditioning and b1
           ride along in the contraction for free.
  stage 2: psum[64, B] += W2p[chunk].T @ tanh_chunk[128, B]
           W2p columns = [shift(32) | log_scale(32)], masked.
Epilogue: exp on ACT, fused multiply/add on DVE writes y back into A.
"""

import sys
import numpy as np

sys.path.insert(0, "/opt/trn_rl_repo")

from contextlib import ExitStack

from concourse import bass, bacc, mybir, tile
from concourse import bass_utils

F32 = mybir.dt.float32
AF = mybir.ActivationFunctionType
ALU = mybir.AluOpType

N, D, H, C = 65536, 32, 1024, 64
NCORES = 8
NS = N // NCORES          # rows per core
KA = D + C + 1            # augmented contraction: y + context + ones
P = 128
HCH = H // P              # 8 hidden chunks


def _masks():
    in_deg = np.arange(1, D + 1)
    hid_deg = np.arange(H) % (D - 1) + 1
    mask1 = (hid_deg[None, :] >= in_deg[:, None]).astype(np.float32)   # [D,H]
    mask2 = (np.arange(1, D + 1)[None, :] > hid_deg[:, None]).astype(np.float32)  # [H,D]
    return mask1, mask2


def _build_dense(ns: int, b: int, with_b2: bool):
    """Dense 32-step kernel. ns = rows per core, b = free-dim block."""
    nc = bacc.Bacc("TRN2", target_bir_lowering=False, debug=False,
                   num_devices=NCORES)
    xt = nc.dram_tensor("xt", [D, ns], F32, kind="ExternalInput")
    a0 = nc.dram_tensor("a0", [KA, ns], F32, kind="ExternalInput")
    wa = nc.dram_tensor("wa", [KA, H], F32, kind="ExternalInput")
    w2 = nc.dram_tensor("w2", [P, HCH * 2 * D], F32, kind="ExternalInput")
    b2r = nc.dram_tensor("b2r", [1, 2 * D], F32, kind="ExternalInput")
    yt = nc.dram_tensor("yt", [D, ns], F32, kind="ExternalOutput")
    lt = nc.dram_tensor("lt", [D, ns], F32, kind="ExternalOutput")

    nblk = ns // b
    with tile.TileContext(nc) as tc, ExitStack() as ctx:
        cpool = ctx.enter_context(tc.tile_pool(name="const", bufs=1))
        apool = ctx.enter_context(tc.tile_pool(name="act", bufs=2))
        hpool = ctx.enter_context(tc.tile_pool(name="hid", bufs=2))
        epool = ctx.enter_context(tc.tile_pool(name="epi", bufs=3))
        psh = ctx.enter_context(
            tc.tile_pool(name="psh", bufs=2, space=bass.MemorySpace.PSUM))
        psp = ctx.enter_context(
            tc.tile_pool(name="psp", bufs=2, space=bass.MemorySpace.PSUM))

        wa_sb = cpool.tile([KA, H], F32)
        w2_sb = cpool.tile([P, HCH * 2 * D], F32)
        nc.sync.dma_start(wa_sb[:], wa[:])
        nc.sync.dma_start(w2_sb[:], w2[:])
        if with_b2:
            b2_sb = cpool.tile([1, 2 * D], F32)
            one_sb = cpool.tile([1, b], F32)
            nc.sync.dma_start(b2_sb[:], b2r[:])
            nc.vector.memset(one_sb[:], 1.0)

        for blk in range(nblk):
            sl = slice(blk * b, (blk + 1) * b)
            A = apool.tile([KA, b], F32)
            X = apool.tile([D, b], F32)
            nc.sync.dma_start(A[:, :], a0[:, sl])
            nc.sync.dma_start(X[:], xt[:, sl])

            for t in range(D):
                hid = hpool.tile([P, HCH * b], F32)
                for q in range(4):          # 4 psum tiles of 2 chunks each
                    ph = psh.tile([P, 2 * b], F32)
                    for j in range(2):
                        k = 2 * q + j
                        nc.tensor.matmul(ph[:, j * b:(j + 1) * b],
                                         wa_sb[:, k * P:(k + 1) * P],
                                         A[:, :], start=True, stop=True)
                    nc.scalar.activation(hid[:, q * 2 * b:(q + 1) * 2 * b],
                                         ph[:, :], AF.Tanh)
                pp = psp.tile([2 * D, b], F32)
                nmm = HCH + (1 if with_b2 else 0)
                for k in range(HCH):
                    nc.tensor.matmul(pp[:, :],
                                     w2_sb[:, k * 2 * D:(k + 1) * 2 * D],
                                     hid[:, k * b:(k + 1) * b],
                                     start=(k == 0), stop=(k == nmm - 1))
                if with_b2:
                    nc.tensor.matmul(pp[:, :], b2_sb[:], one_sb[:],
                                     start=False, stop=True)
                els = epool.tile([D, b], F32)
                nc.scalar.activation(els[:], pp[D:2 * D, :], AF.Exp)
                if t == D - 1:
                    lssb = epool.tile([D, b], F32)
                    nc.vector.tensor_copy(lssb[:], pp[D:2 * D, :])
                    nc.sync.dma_start(lt[:, sl], lssb[:])
                tmul = epool.tile([D, b], F32)
                nc.vector.tensor_tensor(tmul[:], X[:], els[:], ALU.mult)
                nc.vector.tensor_tensor(A[0:D, :], tmul[:], pp[0:D, :],
                                        ALU.add)
            nc.sync.dma_start(yt[:, sl], A[0:D, :])
    nc.compile()
    return nc


def _group_info():
    """Units sorted (stable) by MADE degree; per-degree group extents."""
    hid_deg = np.arange(H) % (D - 1) + 1
    perm = np.argsort(hid_deg, kind="stable")
    sizes = [int(np.sum(hid_deg == g)) for g in range(1, D)]   # g = 1..31
    offs = np.concatenate([[0], np.cumsum(sizes)])
    return perm, sizes, offs


GMAX = 34  # largest degree-group size
MMF = 512   # max fp32 matmul moving-operand free dim


def _build_tri(ns: int, b: int):
    """Triangular kernel: each output dim computed exactly once.

    Per dim d (1..31): one matmul builds the degree-d hidden group's
    pre-activation from A=[y|ctx|1] (masked weights make not-yet-final
    y rows contribute exact zeros), tanh, then one matmul accumulates
    the group's contribution to all 64 output params in a persistent
    PSUM tile (masked W2 adds exact zeros to already-read dims).
    Compute engines require 32-aligned partition bases, so the epilogue
    runs full-width [32, b] (same wall time -- lanes are parallel) and a
    single-row DMA (exempt from the alignment rule) deposits row d into
    A for the next matmul.
    """
    _, sizes, offs = _group_info()
    nc = bacc.Bacc("TRN2", target_bir_lowering=False, debug=False,
                   num_devices=NCORES)
    xt = nc.dram_tensor("xt", [D, ns], F32, kind="ExternalInput")
    a0 = nc.dram_tensor("a0", [KA, ns], F32, kind="ExternalInput")
    wa = nc.dram_tensor("wa", [KA, H], F32, kind="ExternalInput")
    w2g = nc.dram_tensor("w2g", [GMAX, (D - 1) * 2 * D], F32,
                         kind="ExternalInput")
    b2x = nc.dram_tensor("b2x", [D, 3], F32, kind="ExternalInput")
    yt = nc.dram_tensor("yt", [D, ns], F32, kind="ExternalOutput")
    lt = nc.dram_tensor("lt", [D, ns], F32, kind="ExternalOutput")

    nblk = ns // b
    with tile.TileContext(nc) as tc, ExitStack() as ctx:
        cpool = ctx.enter_context(tc.tile_pool(name="const", bufs=1))
        apool = ctx.enter_context(tc.tile_pool(name="act", bufs=2))
        spool = ctx.enter_context(tc.tile_pool(name="stg", bufs=2))
        epool = ctx.enter_context(tc.tile_pool(name="epi", bufs=4))
        ppre = ctx.enter_context(
            tc.tile_pool(name="ppre", bufs=2, space=bass.MemorySpace.PSUM))
        pall = ctx.enter_context(
            tc.tile_pool(name="pall", bufs=2, space=bass.MemorySpace.PSUM))

        wa_sb = cpool.tile([KA, H], F32)
        w2g_sb = cpool.tile([GMAX, (D - 1) * 2 * D], F32)
        b2x_sb = cpool.tile([D, 3], F32)
        nc.sync.dma_start(wa_sb[:], wa[:])
        nc.sync.dma_start(w2g_sb[:], w2g[:])
        nc.sync.dma_start(b2x_sb[:], b2x[:])

        for blk in range(nblk):
            sl = slice(blk * b, (blk + 1) * b)
            A = apool.tile([KA, b], F32)
            X = apool.tile([D, b], F32)
            lsout = apool.tile([D, b], F32)
            nc.sync.dma_start(A[:, :], a0[:, sl])
            nc.sync.dma_start(X[:], xt[:, sl])

            pbig = pall.tile([2 * D, b], F32)
            # dim 0 is precomputed host-side into a0 row 0
            for d in range(1, D):
                sz, u0 = sizes[d - 1], int(offs[d - 1])
                pp = ppre.tile([GMAX, b], F32)
                for f in range(0, b, MMF):
                    fs = slice(f, f + MMF)
                    nc.tensor.matmul(pp[0:sz, fs], wa_sb[:, u0:u0 + sz],
                                     A[:, fs], start=True, stop=True)
                stg = spool.tile([GMAX, b], F32)
                nc.scalar.activation(stg[0:sz, :], pp[0:sz, :], AF.Tanh)
                for f in range(0, b, MMF):
                    fs = slice(f, f + MMF)
                    nc.tensor.matmul(pbig[:, fs],
                                     w2g_sb[0:sz, (d - 1) * 2 * D:d * 2 * D],
                                     stg[0:sz, fs],
                                     start=(d == 1), stop=(d == D - 1),
                                     skip_group_check=True)
                els = epool.tile([D, b], F32)
                nc.scalar.activation(els[:], pbig[D:2 * D, :], AF.Exp,
                                     bias=b2x_sb[0:D, 1:2])
                tm = epool.tile([D, b], F32)
                nc.vector.tensor_tensor(tm[:], X[:], els[:], ALU.mult)
                yn = epool.tile([D, b], F32)
                nc.vector.scalar_tensor_tensor(yn[:], tm[:],
                                               b2x_sb[0:D, 0:1],
                                               pbig[0:D, :],
                                               ALU.add, ALU.add)
                nc.sync.dma_start(A[d:d + 1, :], yn[d:d + 1, :])
            nc.vector.tensor_scalar(lsout[:], pbig[D:2 * D, :],
                                    b2x_sb[0:D, 1:2], None, ALU.add)
            nc.sync.dma_start(yt[:, sl], A[0:D, :])
            nc.sync.dma_start(lt[:, sl], lsout[:])
    nc.compile()
    return nc


def _build_tri2(ns: int, b: int):
    """Triangular kernel v2: dim-major waves of 4 blocks (2 pairs).

    Same math as _build_tri, restructured for engine overlap:
    - No per-dim DMA: the DVE epilogue rewrites A[0:32] full-width each
      dim. Rows <= d are exactly final (later groups add exact zeros to
      their params), rows > d are finite garbage that mask1 zeroes in
      the next matmul.
    - Blocks are processed in waves of 4, dim-major, so the in-order
      engines interleave independent chains.
    - Each pair of blocks shares one [128, b] params PSUM tile: block A
      params at rows 0:64 as [shift|ls], block B at 64:128 with
      column-reversed W2 ([ls|shift]), so one full-tile Exp (base 0,
      legal) and one full-width multiply serve both blocks. The unused
      exp rows (shifts) are computed harmlessly -- ACT time only
      depends on the free size.
    """
    _, sizes, offs = _group_info()
    nc = bacc.Bacc("TRN2", target_bir_lowering=False, debug=False,
                   num_devices=NCORES)
    assert ns % (4 * b) == 0
    xt4 = nc.dram_tensor("xt4", [P, ns // 2], F32, kind="ExternalInput")
    a0 = nc.dram_tensor("a0", [KA, ns], F32, kind="ExternalInput")
    wa = nc.dram_tensor("wa", [KA, H], F32, kind="ExternalInput")
    w2g = nc.dram_tensor("w2g", [GMAX, (D - 1) * 2 * D], F32,
                         kind="ExternalInput")
    w2gr = nc.dram_tensor("w2gr", [GMAX, (D - 1) * 2 * D], F32,
                          kind="ExternalInput")
    b2v = nc.dram_tensor("b2v", [P, 1], F32, kind="ExternalInput")
    b2x = nc.dram_tensor("b2x", [D, 3], F32, kind="ExternalInput")
    yt = nc.dram_tensor("yt", [D, ns], F32, kind="ExternalOutput")
    lt = nc.dram_tensor("lt", [D, ns], F32, kind="ExternalOutput")

    nwave = ns // (4 * b)
    with tile.TileContext(nc) as tc, ExitStack() as ctx:
        cpool = ctx.enter_context(tc.tile_pool(name="const", bufs=1))
        apool = ctx.enter_context(tc.tile_pool(name="act", bufs=8))
        xpool = ctx.enter_context(tc.tile_pool(name="xf", bufs=4))
        spool = ctx.enter_context(tc.tile_pool(name="stg", bufs=4))
        epool = ctx.enter_context(tc.tile_pool(name="epi", bufs=4))
        opool = ctx.enter_context(tc.tile_pool(name="outs", bufs=8))
        pppool = ctx.enter_context(
            tc.tile_pool(name="ppre", bufs=2, space=bass.MemorySpace.PSUM))
        prpool = ctx.enter_context(
            tc.tile_pool(name="pair", bufs=2, space=bass.MemorySpace.PSUM))

        wa_sb = cpool.tile([KA, H], F32)
        w2g_sb = cpool.tile([GMAX, (D - 1) * 2 * D], F32)
        w2gr_sb = cpool.tile([GMAX, (D - 1) * 2 * D], F32)
        b2v_sb = cpool.tile([P, 1], F32)
        b2x_sb = cpool.tile([D, 3], F32)
        nc.sync.dma_start(wa_sb[:], wa[:])
        nc.sync.dma_start(w2g_sb[:], w2g[:])
        nc.sync.dma_start(w2gr_sb[:], w2gr[:])
        nc.sync.dma_start(b2v_sb[:], b2v[:])
        nc.sync.dma_start(b2x_sb[:], b2x[:])

        for w in range(nwave):
            blks = [4 * w + i for i in range(4)]
            A = [apool.tile([KA, b], F32, tag=f"A{i}") for i in range(4)]
            for i in range(4):
                nc.sync.dma_start(A[i][:, :],
                                  a0[:, blks[i] * b:(blks[i] + 1) * b])
            Xf = [xpool.tile([P, b], F32, tag=f"X{i}") for i in range(2)]
            for i in range(2):
                pr = 2 * w + i
                nc.sync.dma_start(Xf[i][:, :], xt4[:, pr * b:(pr + 1) * b])
            pt = [prpool.tile([P, b], F32, tag=f"pt{i}") for i in range(2)]

            for d in range(1, D):
                sz, u0 = sizes[d - 1], int(offs[d - 1])
                gsl = slice((d - 1) * 2 * D, d * 2 * D)
                for h in range(2):
                    pp = []
                    for i in range(2):
                        blk = 2 * i + h
                        t = pppool.tile([GMAX, b], F32, tag=f"pp{i}")
                        pp.append(t)
                        for f in range(0, b, MMF):
                            fs = slice(f, f + MMF)
                            nc.tensor.matmul(t[0:sz, fs],
                                             wa_sb[:, u0:u0 + sz],
                                             A[blk][:, fs],
                                             start=True, stop=True)
                    sg = []
                    for i in range(2):
                        t = spool.tile([GMAX, b], F32, tag=f"sg{i}")
                        sg.append(t)
                        nc.scalar.activation(t[0:sz, :], pp[i][0:sz, :],
                                             AF.Tanh)
                    for i in range(2):
                        wsb = w2g_sb if h == 0 else w2gr_sb
                        for f in range(0, b, MMF):
                            fs = slice(f, f + MMF)
                            nc.tensor.matmul(pt[i][64 * h:64 * h + 64, fs],
                                             wsb[0:sz, gsl],
                                             sg[i][0:sz, fs],
                                             start=(d == 1),
                                             stop=(d == D - 1),
                                             skip_group_check=True)
                els = []
                for i in range(2):
                    t = epool.tile([P, b], F32, tag=f"els{i}")
                    els.append(t)
                    nc.scalar.activation(t[:], pt[i][:, :], AF.Exp,
                                         bias=b2v_sb[:, 0:1])
                tm = []
                for i in range(2):
                    t = epool.tile([P, b], F32, tag=f"tm{i}")
                    tm.append(t)
                    nc.vector.tensor_tensor(t[:], Xf[i][:], els[i][:],
                                            ALU.mult)
                for i in range(2):
                    nc.vector.scalar_tensor_tensor(A[2 * i][0:D, :],
                                                   tm[i][D:2 * D, :],
                                                   b2x_sb[0:D, 0:1],
                                                   pt[i][0:D, :],
                                                   ALU.add, ALU.add)
                    nc.vector.scalar_tensor_tensor(A[2 * i + 1][0:D, :],
                                                   tm[i][2 * D:3 * D, :],
                                                   b2x_sb[0:D, 0:1],
                                                   pt[i][3 * D:4 * D, :],
                                                   ALU.add, ALU.add)
            for i in range(2):
                lsa = opool.tile([D, b], F32, tag=f"lsa{i}")
                lsb = opool.tile([D, b], F32, tag=f"lsb{i}")
                nc.vector.tensor_scalar(lsa[:], pt[i][D:2 * D, :],
                                        b2x_sb[0:D, 1:2], None, ALU.add)
                nc.vector.tensor_scalar(lsb[:], pt[i][2 * D:3 * D, :],
                                        b2x_sb[0:D, 1:2], None, ALU.add)
                nc.sync.dma_start(
                    lt[:, blks[2 * i] * b:(blks[2 * i] + 1) * b], lsa[:])
                nc.sync.dma_start(
                    lt[:, blks[2 * i + 1] * b:(blks[2 * i + 1] + 1) * b],
                    lsb[:])
            for i in range(4):
                nc.sync.dma_start(yt[:, blks[i] * b:(blks[i] + 1) * b],
                                  A[i][0:D, :])
    nc.compile()
    return nc


def _pack_weights_tri(W1, b1, Wc, W2, b2):
    perm, sizes, offs = _group_info()
    Wa, _, _ = _pack_weights(W1, b1, Wc, W2, b2)
    Was = np.ascontiguousarray(Wa[:, perm])
    mask1, mask2 = _masks()
    W2m = (np.asarray(W2) * mask2[:, :, None]).astype(np.float32)
    W2p = np.concatenate([W2m[:, :, 0], W2m[:, :, 1]], 1)[perm]   # sorted
    w2g = np.zeros((GMAX, (D - 1) * 2 * D), np.float32)
    for g in range(1, D):
        sz, u0 = sizes[g - 1], int(offs[g - 1])
        w2g[0:sz, (g - 1) * 2 * D:g * 2 * D] = W2p[u0:u0 + sz, :]
    b2 = np.asarray(b2, np.float32)
    b2x = np.zeros((D, 3), np.float32)
    b2x[:, 0] = b2[:, 0]
    b2x[:, 1] = b2[:, 1]
    b2x[0, 2] = np.exp(b2[0, 1])
    return Was, w2g, b2x

def _pack_weights(W1, b1, Wc, W2, b2):
    mask1, mask2 = _masks()
    W1m = (np.asarray(W1) * mask1).astype(np.float32)
    W2m = (np.asarray(W2) * mask2[:, :, None]).astype(np.float32)
    Wa = np.concatenate([W1m, np.asarray(Wc, np.float32),
                         np.asarray(b1, np.float32)[None, :]], 0)   # [97,H]
    W2p = np.concatenate([W2m[:, :, 0], W2m[:, :, 1]], 1)           # [H,64]
    w2pack = np.ascontiguousarray(
        W2p.reshape(HCH, P, 2 * D).transpose(1, 0, 2).reshape(P, HCH * 2 * D))
    b2p = np.concatenate([np.asarray(b2, np.float32)[:, 0],
                          np.asarray(b2, np.float32)[:, 1]])[None, :]  # [1,64]
    return np.ascontiguousarray(Wa), w2pack, b2p


_CACHE = {}


def _get_nc(key, builder):
    if key not in _CACHE:
        _CACHE[key] = builder()
    return _CACHE[key]


B_TRI = 1024


def kernel(x, context, W1, b1, Wc, W2, b2):
    x = np.asarray(x, np.float32)
    context = np.asarray(context, np.float32)
    b2 = np.asarray(b2, np.float32)
    Was, w2g, b2x = _pack_weights_tri(W1, b1, Wc, W2, b2)

    nc = _get_nc(("tri", NS, B_TRI), lambda: _build_tri(NS, B_TRI))

    in_maps = []
    for c in range(NCORES):
        rows = slice(c * NS, (c + 1) * NS)
        a0 = np.empty((KA, NS), np.float32)
        a0[0:D] = 0.0
        a0[0] = x[rows, 0] * np.exp(b2[0, 1]) + b2[0, 0]
        a0[D:D + C] = context[rows].T
        a0[D + C:] = 1.0
        in_maps.append({
            "xt": np.ascontiguousarray(x[rows].T),
            "a0": a0,
            "wa": Was, "w2g": w2g, "b2x": b2x,
        })
    res = bass_utils.run_bass_kernel_spmd(nc, in_maps,
                                          core_ids=list(range(NCORES)))
    global LAST_RESULT, LAST_IN_MAPS
    LAST_RESULT = res
    LAST_IN_MAPS = in_maps
    y = np.empty((N, D), np.float32)
    ld = np.empty((N,), np.float32)
    for c, out in enumerate(res.results):
        rows = slice(c * NS, (c + 1) * NS)
        y[rows] = out["yt"].T
        ld[rows] = out["lt"].sum(axis=0)
    return y, ld


# revision 28
# speedup vs baseline: 1189.3131x; 1189.3131x over previous
"""Trainium2 Bass kernel for conditional MAF (MADE) forward pass.

Reference computation (N=65536, D=32, H=1024, C=64):
    Wc_h = context @ Wc
    repeat D times:  hid = tanh(y @ (W1*mask1) + Wc_h + b1)
                     params = hid @ (W2*mask2) + b2        # [N, D, 2]
                     y = x * exp(params[...,1]) + params[...,0]
    returns (y, params[...,1].sum(-1))

Strategy: pure data-parallel over 8 NeuronCores (8192 rows each).
On-chip layout keeps the batch on the free dimension (activations
transposed) so both matmul stages contract over partitions:
  stage 1: psum[128u, B] = Wa[:, chunk].T @ A[97, B]
           where A rows = [y (32) | context (64) | ones (1)] and
           Wa = [W1*mask1 ; Wc ; b1] -- context conditioning and b1
           ride along in the contraction for free.
  stage 2: psum[64, B] += W2p[chunk].T @ tanh_chunk[128, B]
           W2p columns = [shift(32) | log_scale(32)], masked.
Epilogue: exp on ACT, fused multiply/add on DVE writes y back into A.
"""

import sys
import numpy as np

sys.path.insert(0, "/opt/trn_rl_repo")

from contextlib import ExitStack

from concourse import bass, bacc, mybir, tile
from concourse import bass_utils

F32 = mybir.dt.float32
AF = mybir.ActivationFunctionType
ALU = mybir.AluOpType

N, D, H, C = 65536, 32, 1024, 64
NCORES = 8
NS = N // NCORES          # rows per core
KA = D + C + 1            # augmented contraction: y + context + ones
P = 128
HCH = H // P              # 8 hidden chunks


def _masks():
    in_deg = np.arange(1, D + 1)
    hid_deg = np.arange(H) % (D - 1) + 1
    mask1 = (hid_deg[None, :] >= in_deg[:, None]).astype(np.float32)   # [D,H]
    mask2 = (np.arange(1, D + 1)[None, :] > hid_deg[:, None]).astype(np.float32)  # [H,D]
    return mask1, mask2


def _build_dense(ns: int, b: int, with_b2: bool):
    """Dense 32-step kernel. ns = rows per core, b = free-dim block."""
    nc = bacc.Bacc("TRN2", target_bir_lowering=False, debug=False,
                   num_devices=NCORES)
    xt = nc.dram_tensor("xt", [D, ns], F32, kind="ExternalInput")
    a0 = nc.dram_tensor("a0", [KA, ns], F32, kind="ExternalInput")
    wa = nc.dram_tensor("wa", [KA, H], F32, kind="ExternalInput")
    w2 = nc.dram_tensor("w2", [P, HCH * 2 * D], F32, kind="ExternalInput")
    b2r = nc.dram_tensor("b2r", [1, 2 * D], F32, kind="ExternalInput")
    yt = nc.dram_tensor("yt", [D, ns], F32, kind="ExternalOutput")
    lt = nc.dram_tensor("lt", [D, ns], F32, kind="ExternalOutput")

    nblk = ns // b
    with tile.TileContext(nc) as tc, ExitStack() as ctx:
        cpool = ctx.enter_context(tc.tile_pool(name="const", bufs=1))
        apool = ctx.enter_context(tc.tile_pool(name="act", bufs=2))
        hpool = ctx.enter_context(tc.tile_pool(name="hid", bufs=2))
        epool = ctx.enter_context(tc.tile_pool(name="epi", bufs=3))
        psh = ctx.enter_context(
            tc.tile_pool(name="psh", bufs=2, space=bass.MemorySpace.PSUM))
        psp = ctx.enter_context(
            tc.tile_pool(name="psp", bufs=2, space=bass.MemorySpace.PSUM))

        wa_sb = cpool.tile([KA, H], F32)
        w2_sb = cpool.tile([P, HCH * 2 * D], F32)
        nc.sync.dma_start(wa_sb[:], wa[:])
        nc.sync.dma_start(w2_sb[:], w2[:])
        if with_b2:
            b2_sb = cpool.tile([1, 2 * D], F32)
            one_sb = cpool.tile([1, b], F32)
            nc.sync.dma_start(b2_sb[:], b2r[:])
            nc.vector.memset(one_sb[:], 1.0)

        for blk in range(nblk):
            sl = slice(blk * b, (blk + 1) * b)
            A = apool.tile([KA, b], F32)
            X = apool.tile([D, b], F32)
            nc.sync.dma_start(A[:, :], a0[:, sl])
            nc.sync.dma_start(X[:], xt[:, sl])

            for t in range(D):
                hid = hpool.tile([P, HCH * b], F32)
                for q in range(4):          # 4 psum tiles of 2 chunks each
                    ph = psh.tile([P, 2 * b], F32)
                    for j in range(2):
                        k = 2 * q + j
                        nc.tensor.matmul(ph[:, j * b:(j + 1) * b],
                                         wa_sb[:, k * P:(k + 1) * P],
                                         A[:, :], start=True, stop=True)
                    nc.scalar.activation(hid[:, q * 2 * b:(q + 1) * 2 * b],
                                         ph[:, :], AF.Tanh)
                pp = psp.tile([2 * D, b], F32)
                nmm = HCH + (1 if with_b2 else 0)
                for k in range(HCH):
                    nc.tensor.matmul(pp[:, :],
                                     w2_sb[:, k * 2 * D:(k + 1) * 2 * D],
                                     hid[:, k * b:(k + 1) * b],
                                     start=(k == 0), stop=(k == nmm - 1))
                if with_b2:
                    nc.tensor.matmul(pp[:, :], b2_sb[:], one_sb[:],
                                     start=False, stop=True)
                els = epool.tile([D, b], F32)
                nc.scalar.activation(els[:], pp[D:2 * D, :], AF.Exp)
                if t == D - 1:
                    lssb = epool.tile([D, b], F32)
                    nc.vector.tensor_copy(lssb[:], pp[D:2 * D, :])
                    nc.sync.dma_start(lt[:, sl], lssb[:])
                tmul = epool.tile([D, b], F32)
                nc.vector.tensor_tensor(tmul[:], X[:], els[:], ALU.mult)
                nc.vector.tensor_tensor(A[0:D, :], tmul[:], pp[0:D, :],
                                        ALU.add)
            nc.sync.dma_start(yt[:, sl], A[0:D, :])
    nc.compile()
    return nc


def _group_info():
    """Units sorted (stable) by MADE degree; per-degree group extents."""
    hid_deg = np.arange(H) % (D - 1) + 1
    perm = np.argsort(hid_deg, kind="stable")
    sizes = [int(np.sum(hid_deg == g)) for g in range(1, D)]   # g = 1..31
    offs = np.concatenate([[0], np.cumsum(sizes)])
    return perm, sizes, offs


GMAX = 34  # largest degree-group size
MMF = 512   # max fp32 matmul moving-operand free dim


def _build_tri(ns: int, b: int):
    """Triangular kernel: each output dim computed exactly once.

    Per dim d (1..31): one matmul builds the degree-d hidden group's
    pre-activation from A=[y|ctx|1] (masked weights make not-yet-final
    y rows contribute exact zeros), tanh, then one matmul accumulates
    the group's contribution to all 64 output params in a persistent
    PSUM tile (masked W2 adds exact zeros to already-read dims).
    Compute engines require 32-aligned partition bases, so the epilogue
    runs full-width [32, b] (same wall time -- lanes are parallel) and a
    single-row DMA (exempt from the alignment rule) deposits row d into
    A for the next matmul.
    """
    _, sizes, offs = _group_info()
    nc = bacc.Bacc("TRN2", target_bir_lowering=False, debug=False,
                   num_devices=NCORES)
    xt = nc.dram_tensor("xt", [D, ns], F32, kind="ExternalInput")
    a0 = nc.dram_tensor("a0", [KA, ns], F32, kind="ExternalInput")
    wa = nc.dram_tensor("wa", [KA, H], F32, kind="ExternalInput")
    w2g = nc.dram_tensor("w2g", [GMAX, (D - 1) * 2 * D], F32,
                         kind="ExternalInput")
    b2x = nc.dram_tensor("b2x", [D, 3], F32, kind="ExternalInput")
    yt = nc.dram_tensor("yt", [D, ns], F32, kind="ExternalOutput")
    lt = nc.dram_tensor("lt", [D, ns], F32, kind="ExternalOutput")

    nblk = ns // b
    with tile.TileContext(nc) as tc, ExitStack() as ctx:
        cpool = ctx.enter_context(tc.tile_pool(name="const", bufs=1))
        apool = ctx.enter_context(tc.tile_pool(name="act", bufs=2))
        spool = ctx.enter_context(tc.tile_pool(name="stg", bufs=2))
        epool = ctx.enter_context(tc.tile_pool(name="epi", bufs=4))
        ppre = ctx.enter_context(
            tc.tile_pool(name="ppre", bufs=2, space=bass.MemorySpace.PSUM))
        pall = ctx.enter_context(
            tc.tile_pool(name="pall", bufs=2, space=bass.MemorySpace.PSUM))

        wa_sb = cpool.tile([KA, H], F32)
        w2g_sb = cpool.tile([GMAX, (D - 1) * 2 * D], F32)
        b2x_sb = cpool.tile([D, 3], F32)
        nc.sync.dma_start(wa_sb[:], wa[:])
        nc.sync.dma_start(w2g_sb[:], w2g[:])
        nc.sync.dma_start(b2x_sb[:], b2x[:])

        for blk in range(nblk):
            sl = slice(blk * b, (blk + 1) * b)
            A = apool.tile([KA, b], F32)
            X = apool.tile([D, b], F32)
            lsout = apool.tile([D, b], F32)
            nc.sync.dma_start(A[:, :], a0[:, sl])
            nc.sync.dma_start(X[:], xt[:, sl])

            pbig = pall.tile([2 * D, b], F32)
            # dim 0 is precomputed host-side into a0 row 0
            for d in range(1, D):
                sz, u0 = sizes[d - 1], int(offs[d - 1])
                pp = ppre.tile([GMAX, b], F32)
                for f in range(0, b, MMF):
                    fs = slice(f, f + MMF)
                    nc.tensor.matmul(pp[0:sz, fs], wa_sb[:, u0:u0 + sz],
                                     A[:, fs], start=True, stop=True)
                stg = spool.tile([GMAX, b], F32)
                nc.scalar.activation(stg[0:sz, :], pp[0:sz, :], AF.Tanh)
                for f in range(0, b, MMF):
                    fs = slice(f, f + MMF)
                    nc.tensor.matmul(pbig[:, fs],
                                     w2g_sb[0:sz, (d - 1) * 2 * D:d * 2 * D],
                                     stg[0:sz, fs],
                                     start=(d == 1), stop=(d == D - 1),
                                     skip_group_check=True)
                els = epool.tile([D, b], F32)
                nc.scalar.activation(els[:], pbig[D:2 * D, :], AF.Exp,
                                     bias=b2x_sb[0:D, 1:2])
                tm = epool.tile([D, b], F32)
                nc.vector.tensor_tensor(tm[:], X[:], els[:], ALU.mult)
                yn = epool.tile([D, b], F32)
                nc.vector.scalar_tensor_tensor(yn[:], tm[:],
                                               b2x_sb[0:D, 0:1],
                                               pbig[0:D, :],
                                               ALU.add, ALU.add)
                nc.sync.dma_start(A[d:d + 1, :], yn[d:d + 1, :])
            nc.vector.tensor_scalar(lsout[:], pbig[D:2 * D, :],
                                    b2x_sb[0:D, 1:2], None, ALU.add)
            nc.sync.dma_start(yt[:, sl], A[0:D, :])
            nc.sync.dma_start(lt[:, sl], lsout[:])
    nc.compile()
    return nc


def _build_tri2(ns: int, b: int, chain: bool = True, use_f32r: bool = False):
    """Triangular kernel v3: f32r matmuls (fp32 bits, full-rate PE mode;
    requires tile_position (0,0), so no partition packing), dim-major
    waves of 2 blocks, no per-dim DMA (DVE rewrites A[0:32] full-width;
    rows <= d are exactly final, rows > d finite garbage zeroed by
    mask1 in the next matmul)."""
    F32R = mybir.dt.float32r if use_f32r else F32
    _, sizes, offs = _group_info()
    nc = bacc.Bacc("TRN2", target_bir_lowering=False, debug=False,
                   num_devices=NCORES)
    assert ns % (2 * b) == 0
    xt2 = nc.dram_tensor("xt2", [2 * D, ns], F32, kind="ExternalInput")
    a0 = nc.dram_tensor("a0", [KA, ns], F32R, kind="ExternalInput")
    wa = nc.dram_tensor("wa", [KA, H], F32R, kind="ExternalInput")
    w2b = nc.dram_tensor("w2b", [P, (D - 1) * 2 * D], F32R,
                         kind="ExternalInput")
    b2v = nc.dram_tensor("b2v", [P, 2], F32, kind="ExternalInput")
    b2x = nc.dram_tensor("b2x", [D, 3], F32, kind="ExternalInput")
    yt = nc.dram_tensor("yt", [D, ns], F32, kind="ExternalOutput")
    lt = nc.dram_tensor("lt", [D, ns], F32, kind="ExternalOutput")

    nwave = ns // (2 * b)
    with tile.TileContext(nc) as tc, ExitStack() as ctx:
        cpool = ctx.enter_context(tc.tile_pool(name="const", bufs=1))
        apool = ctx.enter_context(tc.tile_pool(name="act", bufs=2))
        xpool = ctx.enter_context(tc.tile_pool(name="xf", bufs=2))
        spool = ctx.enter_context(tc.tile_pool(name="stg", bufs=2))
        epool = ctx.enter_context(tc.tile_pool(name="epi", bufs=2))
        opool = ctx.enter_context(tc.tile_pool(name="outs", bufs=2))
        pppool = ctx.enter_context(
            tc.tile_pool(name="ppre", bufs=1, space=bass.MemorySpace.PSUM))
        prpool = ctx.enter_context(
            tc.tile_pool(name="pt", bufs=1, space=bass.MemorySpace.PSUM))

        wa_sb = cpool.tile([KA, H], F32R)
        w2b_sb = cpool.tile([P, (D - 1) * 2 * D], F32R)
        b2v_sb = cpool.tile([P, 2], F32)
        b2x_sb = cpool.tile([D, 3], F32)
        nc.sync.dma_start(wa_sb[:], wa[:])
        nc.sync.dma_start(w2b_sb[:], w2b[:])
        nc.sync.dma_start(b2v_sb[:], b2v[:])
        nc.sync.dma_start(b2x_sb[:], b2x[:])

        for w in range(nwave):
            blks = [2 * w + i for i in range(2)]
            A = [apool.tile([KA, b], F32R, tag=f"A{i}", name=f"A{w}_{i}")
                 for i in range(2)]
            Xf = [xpool.tile([2 * D, b], F32, tag=f"X{i}", name=f"Xf{w}_{i}")
                  for i in range(2)]
            pt = [prpool.tile([2 * D, b], F32, tag=f"pt{i}", name=f"pt{w}_{i}")
                  for i in range(2)]
            for i in range(2):
                nc.sync.dma_start(A[i][:, :],
                                  a0[:, blks[i] * b:(blks[i] + 1) * b])
                nc.sync.dma_start(Xf[i][:, :],
                                  xt2[:, blks[i] * b:(blks[i] + 1) * b])

            for d in range(1, D):
                sz, u0 = sizes[d - 1], int(offs[d - 1])
                gsl = slice((d - 1) * 2 * D, d * 2 * D)
                pp = [pppool.tile([GMAX, b], F32, tag=f"pp{i}",
                                  name=f"pp{w}_{d}_{i}") for i in range(2)]
                for i in range(2):
                    for f in range(0, b, MMF):
                        fs = slice(f, f + MMF)
                        nc.tensor.matmul(pp[i][0:sz, fs],
                                         wa_sb[:, u0:u0 + sz],
                                         A[i][:, fs], start=True, stop=True)
                sg = [spool.tile([GMAX, b], F32R, tag=f"sg{i}",
                                 name=f"sg{w}_{d}_{i}") for i in range(2)]
                for i in range(2):
                    nc.scalar.activation(sg[i][0:sz, :], pp[i][0:sz, :],
                                         AF.Tanh)
                for i in range(2):
                    for f in range(0, b, MMF):
                        fs = slice(f, f + MMF)
                        nc.tensor.matmul(pt[i][:, fs],
                                         w2b_sb[0:sz, gsl],
                                         sg[i][0:sz, fs],
                                         start=(d == 1), stop=(d == D - 1),
                                         skip_group_check=True)
                els = [epool.tile([2 * D, b], F32, tag=f"els{i}",
                                  name=f"els{w}_{d}_{i}") for i in range(2)]
                tm = [epool.tile([2 * D, b], F32, tag=f"tm{i}",
                                 name=f"tm{w}_{d}_{i}") for i in range(2)]
                for i in range(2):
                    nc.scalar.activation(els[i][:], pt[i][:, :], AF.Exp,
                                         bias=b2v_sb[0:2 * D, 0:1])
                for i in range(2):
                    nc.vector.tensor_tensor(tm[i][:], Xf[i][:], els[i][:],
                                            ALU.mult)
                for i in range(2):
                    out = A[i][0:D, :] if chain else \
                        epool.tile([D, b], F32, tag=f"sc{i}",
                                   name=f"sc{w}_{d}_{i}")[:]
                    nc.vector.scalar_tensor_tensor(out,
                                                   tm[i][D:2 * D, :],
                                                   b2v_sb[D:2 * D, 1:2],
                                                   pt[i][0:D, :],
                                                   ALU.add, ALU.add)
            for i in range(2):
                lso = opool.tile([D, b], F32, tag=f"ls{i}", name=f"ls{w}_{i}")
                yout = opool.tile([D, b], F32, tag=f"yo{i}", name=f"yo{w}_{i}")
                nc.vector.tensor_scalar(lso[:], pt[i][D:2 * D, :],
                                        b2x_sb[0:D, 1:2], None, ALU.add)
                nc.vector.tensor_copy(yout[:], A[i][0:D, :])
                nc.sync.dma_start(lt[:, blks[i] * b:(blks[i] + 1) * b],
                                  lso[:])
                nc.sync.dma_start(yt[:, blks[i] * b:(blks[i] + 1) * b],
                                  yout[:])
    nc.compile()
    return nc


def _pack_weights(W1, b1, Wc, W2, b2):
    mask1, mask2 = _masks()
    W1m = (np.asarray(W1) * mask1).astype(np.float32)
    W2m = (np.asarray(W2) * mask2[:, :, None]).astype(np.float32)
    Wa = np.concatenate([W1m, np.asarray(Wc, np.float32),
                         np.asarray(b1, np.float32)[None, :]], 0)   # [97,H]
    W2p = np.concatenate([W2m[:, :, 0], W2m[:, :, 1]], 1)           # [H,64]
    w2pack = np.ascontiguousarray(
        W2p.reshape(HCH, P, 2 * D).transpose(1, 0, 2).reshape(P, HCH * 2 * D))
    b2p = np.concatenate([np.asarray(b2, np.float32)[:, 0],
                          np.asarray(b2, np.float32)[:, 1]])[None, :]  # [1,64]
    return np.ascontiguousarray(Wa), w2pack, b2p


def _pack_weights_tri(W1, b1, Wc, W2, b2):
    perm, sizes, offs = _group_info()
    Wa, _, _ = _pack_weights(W1, b1, Wc, W2, b2)
    Was = np.ascontiguousarray(Wa[:, perm])
    mask1, mask2 = _masks()
    W2m = (np.asarray(W2) * mask2[:, :, None]).astype(np.float32)
    W2p = np.concatenate([W2m[:, :, 0], W2m[:, :, 1]], 1)[perm]   # sorted
    w2g = np.zeros((GMAX, (D - 1) * 2 * D), np.float32)
    for g in range(1, D):
        sz, u0 = sizes[g - 1], int(offs[g - 1])
        w2g[0:sz, (g - 1) * 2 * D:g * 2 * D] = W2p[u0:u0 + sz, :]
    b2 = np.asarray(b2, np.float32)
    b2x = np.zeros((D, 3), np.float32)
    b2x[:, 0] = b2[:, 0]
    b2x[:, 1] = b2[:, 1]
    b2x[0, 2] = np.exp(b2[0, 1])
    return Was, w2g, b2x


def _pack_weights_tri2(W1, b1, Wc, W2, b2, ns):
    perm, sizes, offs = _group_info()
    Was, w2g, b2x = _pack_weights_tri(W1, b1, Wc, W2, b2)
    mask1, mask2 = _masks()
    W2m = (np.asarray(W2) * mask2[:, :, None]).astype(np.float32)
    w2b = np.zeros((P, (D - 1) * 2 * D), np.float32)
    w2b[0:GMAX, :] = w2g
    b2 = np.asarray(b2, np.float32)
    b2v = np.zeros((P, 2), np.float32)
    b2v[D:2 * D, 0] = b2[:, 1]
    b2v[2 * D:3 * D, 0] = b2[:, 1]
    b2v[D:2 * D, 1] = b2[:, 0]
    b2v[2 * D:3 * D, 1] = b2[:, 0]
    return Was, w2b, b2v, b2x


def _pack_x2(x_shard_T):
    """xt2 [64, ns]: rows 32:64 = x dims, rest zero."""
    ns = x_shard_T.shape[1]
    xt2 = np.zeros((2 * D, ns), np.float32)
    xt2[D:2 * D, :] = x_shard_T
    return xt2


_CACHE = {}


def _get_nc(key, builder):
    if key not in _CACHE:
        _CACHE[key] = builder()
    return _CACHE[key]


B_TRI = 1024


def kernel(x, context, W1, b1, Wc, W2, b2):
    x = np.asarray(x, np.float32)
    context = np.asarray(context, np.float32)
    b2 = np.asarray(b2, np.float32)
    Was, w2b, b2v, b2x = _pack_weights_tri2(W1, b1, Wc, W2, b2, NS)

    nc = _get_nc(("tri3", NS, B_TRI), lambda: _build_tri2(NS, B_TRI))

    in_maps = []
    for c in range(NCORES):
        rows = slice(c * NS, (c + 1) * NS)
        a0 = np.empty((KA, NS), np.float32)
        a0[0:D] = 0.0
        a0[0] = x[rows, 0] * np.exp(b2[0, 1]) + b2[0, 0]
        a0[D:D + C] = context[rows].T
        a0[D + C:] = 1.0
        in_maps.append({
            "xt2": _pack_x2(np.ascontiguousarray(x[rows].T)),
            "a0": a0,
            "wa": Was, "w2b": w2b, "b2v": b2v, "b2x": b2x,
        })
    res = bass_utils.run_bass_kernel_spmd(nc, in_maps,
                                          core_ids=list(range(NCORES)))
    global LAST_RESULT, LAST_IN_MAPS
    LAST_RESULT = res
    LAST_IN_MAPS = in_maps
    y = np.empty((N, D), np.float32)
    ld = np.empty((N,), np.float32)
    for c, out in enumerate(res.results):
        rows = slice(c * NS, (c + 1) * NS)
        y[rows] = out["yt"].T
        ld[rows] = out["lt"].sum(axis=0)
    return y, ld


# revision 30
# speedup vs baseline: 1583.6666x; 1.3316x over previous
"""Trainium2 Bass kernel for conditional MAF (MADE) forward pass.

Reference computation (N=65536, D=32, H=1024, C=64):
    Wc_h = context @ Wc
    repeat D times:  hid = tanh(y @ (W1*mask1) + Wc_h + b1)
                     params = hid @ (W2*mask2) + b2        # [N, D, 2]
                     y = x * exp(params[...,1]) + params[...,0]
    returns (y, params[...,1].sum(-1))

Strategy: pure data-parallel over 8 NeuronCores (8192 rows each).
On-chip layout keeps the batch on the free dimension (activations
transposed) so both matmul stages contract over partitions:
  stage 1: psum[128u, B] = Wa[:, chunk].T @ A[97, B]
           where A rows = [y (32) | context (64) | ones (1)] and
           Wa = [W1*mask1 ; Wc ; b1] -- context conditioning and b1
           ride along in the contraction for free.
  stage 2: psum[64, B] += W2p[chunk].T @ tanh_chunk[128, B]
           W2p columns = [shift(32) | log_scale(32)], masked.
Epilogue: exp on ACT, fused multiply/add on DVE writes y back into A.
"""

import sys
import numpy as np

sys.path.insert(0, "/opt/trn_rl_repo")

from contextlib import ExitStack

from concourse import bass, bacc, mybir, tile
from concourse import bass_utils

F32 = mybir.dt.float32
AF = mybir.ActivationFunctionType
ALU = mybir.AluOpType

N, D, H, C = 65536, 32, 1024, 64
NCORES = 8
NS = N // NCORES          # rows per core
KA = D + C + 1            # augmented contraction: y + context + ones
P = 128
HCH = H // P              # 8 hidden chunks


def _masks():
    in_deg = np.arange(1, D + 1)
    hid_deg = np.arange(H) % (D - 1) + 1
    mask1 = (hid_deg[None, :] >= in_deg[:, None]).astype(np.float32)   # [D,H]
    mask2 = (np.arange(1, D + 1)[None, :] > hid_deg[:, None]).astype(np.float32)  # [H,D]
    return mask1, mask2


def _build_dense(ns: int, b: int, with_b2: bool):
    """Dense 32-step kernel. ns = rows per core, b = free-dim block."""
    nc = bacc.Bacc("TRN2", target_bir_lowering=False, debug=False,
                   num_devices=NCORES)
    xt = nc.dram_tensor("xt", [D, ns], F32, kind="ExternalInput")
    a0 = nc.dram_tensor("a0", [KA, ns], F32, kind="ExternalInput")
    wa = nc.dram_tensor("wa", [KA, H], F32, kind="ExternalInput")
    w2 = nc.dram_tensor("w2", [P, HCH * 2 * D], F32, kind="ExternalInput")
    b2r = nc.dram_tensor("b2r", [1, 2 * D], F32, kind="ExternalInput")
    yt = nc.dram_tensor("yt", [D, ns], F32, kind="ExternalOutput")
    lt = nc.dram_tensor("lt", [D, ns], F32, kind="ExternalOutput")

    nblk = ns // b
    with tile.TileContext(nc) as tc, ExitStack() as ctx:
        cpool = ctx.enter_context(tc.tile_pool(name="const", bufs=1))
        apool = ctx.enter_context(tc.tile_pool(name="act", bufs=2))
        hpool = ctx.enter_context(tc.tile_pool(name="hid", bufs=2))
        epool = ctx.enter_context(tc.tile_pool(name="epi", bufs=3))
        psh = ctx.enter_context(
            tc.tile_pool(name="psh", bufs=2, space=bass.MemorySpace.PSUM))
        psp = ctx.enter_context(
            tc.tile_pool(name="psp", bufs=2, space=bass.MemorySpace.PSUM))

        wa_sb = cpool.tile([KA, H], F32)
        w2_sb = cpool.tile([P, HCH * 2 * D], F32)
        nc.sync.dma_start(wa_sb[:], wa[:])
        nc.sync.dma_start(w2_sb[:], w2[:])
        if with_b2:
            b2_sb = cpool.tile([1, 2 * D], F32)
            one_sb = cpool.tile([1, b], F32)
            nc.sync.dma_start(b2_sb[:], b2r[:])
            nc.vector.memset(one_sb[:], 1.0)

        for blk in range(nblk):
            sl = slice(blk * b, (blk + 1) * b)
            A = apool.tile([KA, b], F32)
            X = apool.tile([D, b], F32)
            nc.sync.dma_start(A[:, :], a0[:, sl])
            nc.sync.dma_start(X[:], xt[:, sl])

            for t in range(D):
                hid = hpool.tile([P, HCH * b], F32)
                for q in range(4):          # 4 psum tiles of 2 chunks each
                    ph = psh.tile([P, 2 * b], F32)
                    for j in range(2):
                        k = 2 * q + j
                        nc.tensor.matmul(ph[:, j * b:(j + 1) * b],
                                         wa_sb[:, k * P:(k + 1) * P],
                                         A[:, :], start=True, stop=True)
                    nc.scalar.activation(hid[:, q * 2 * b:(q + 1) * 2 * b],
                                         ph[:, :], AF.Tanh)
                pp = psp.tile([2 * D, b], F32)
                nmm = HCH + (1 if with_b2 else 0)
                for k in range(HCH):
                    nc.tensor.matmul(pp[:, :],
                                     w2_sb[:, k * 2 * D:(k + 1) * 2 * D],
                                     hid[:, k * b:(k + 1) * b],
                                     start=(k == 0), stop=(k == nmm - 1))
                if with_b2:
                    nc.tensor.matmul(pp[:, :], b2_sb[:], one_sb[:],
                                     start=False, stop=True)
                els = epool.tile([D, b], F32)
                nc.scalar.activation(els[:], pp[D:2 * D, :], AF.Exp)
                if t == D - 1:
                    lssb = epool.tile([D, b], F32)
                    nc.vector.tensor_copy(lssb[:], pp[D:2 * D, :])
                    nc.sync.dma_start(lt[:, sl], lssb[:])
                tmul = epool.tile([D, b], F32)
                nc.vector.tensor_tensor(tmul[:], X[:], els[:], ALU.mult)
                nc.vector.tensor_tensor(A[0:D, :], tmul[:], pp[0:D, :],
                                        ALU.add)
            nc.sync.dma_start(yt[:, sl], A[0:D, :])
    nc.compile()
    return nc


def _group_info():
    """Units sorted (stable) by MADE degree; per-degree group extents."""
    hid_deg = np.arange(H) % (D - 1) + 1
    perm = np.argsort(hid_deg, kind="stable")
    sizes = [int(np.sum(hid_deg == g)) for g in range(1, D)]   # g = 1..31
    offs = np.concatenate([[0], np.cumsum(sizes)])
    return perm, sizes, offs


GMAX = 34  # largest degree-group size
MMF = 512   # max fp32 matmul moving-operand free dim


def _build_tri(ns: int, b: int):
    """Triangular kernel: each output dim computed exactly once.

    Per dim d (1..31): one matmul builds the degree-d hidden group's
    pre-activation from A=[y|ctx|1] (masked weights make not-yet-final
    y rows contribute exact zeros), tanh, then one matmul accumulates
    the group's contribution to all 64 output params in a persistent
    PSUM tile (masked W2 adds exact zeros to already-read dims).
    Compute engines require 32-aligned partition bases, so the epilogue
    runs full-width [32, b] (same wall time -- lanes are parallel) and a
    single-row DMA (exempt from the alignment rule) deposits row d into
    A for the next matmul.
    """
    _, sizes, offs = _group_info()
    nc = bacc.Bacc("TRN2", target_bir_lowering=False, debug=False,
                   num_devices=NCORES)
    xt = nc.dram_tensor("xt", [D, ns], F32, kind="ExternalInput")
    a0 = nc.dram_tensor("a0", [KA, ns], F32, kind="ExternalInput")
    wa = nc.dram_tensor("wa", [KA, H], F32, kind="ExternalInput")
    w2g = nc.dram_tensor("w2g", [GMAX, (D - 1) * 2 * D], F32,
                         kind="ExternalInput")
    b2x = nc.dram_tensor("b2x", [D, 3], F32, kind="ExternalInput")
    yt = nc.dram_tensor("yt", [D, ns], F32, kind="ExternalOutput")
    lt = nc.dram_tensor("lt", [D, ns], F32, kind="ExternalOutput")

    nblk = ns // b
    with tile.TileContext(nc) as tc, ExitStack() as ctx:
        cpool = ctx.enter_context(tc.tile_pool(name="const", bufs=1))
        apool = ctx.enter_context(tc.tile_pool(name="act", bufs=2))
        spool = ctx.enter_context(tc.tile_pool(name="stg", bufs=2))
        epool = ctx.enter_context(tc.tile_pool(name="epi", bufs=4))
        ppre = ctx.enter_context(
            tc.tile_pool(name="ppre", bufs=2, space=bass.MemorySpace.PSUM))
        pall = ctx.enter_context(
            tc.tile_pool(name="pall", bufs=2, space=bass.MemorySpace.PSUM))

        wa_sb = cpool.tile([KA, H], F32)
        w2g_sb = cpool.tile([GMAX, (D - 1) * 2 * D], F32)
        b2x_sb = cpool.tile([D, 3], F32)
        nc.sync.dma_start(wa_sb[:], wa[:])
        nc.sync.dma_start(w2g_sb[:], w2g[:])
        nc.sync.dma_start(b2x_sb[:], b2x[:])

        for blk in range(nblk):
            sl = slice(blk * b, (blk + 1) * b)
            A = apool.tile([KA, b], F32)
            X = apool.tile([D, b], F32)
            lsout = apool.tile([D, b], F32)
            nc.sync.dma_start(A[:, :], a0[:, sl])
            nc.sync.dma_start(X[:], xt[:, sl])

            pbig = pall.tile([2 * D, b], F32)
            # dim 0 is precomputed host-side into a0 row 0
            for d in range(1, D):
                sz, u0 = sizes[d - 1], int(offs[d - 1])
                pp = ppre.tile([GMAX, b], F32)
                for f in range(0, b, MMF):
                    fs = slice(f, f + MMF)
                    nc.tensor.matmul(pp[0:sz, fs], wa_sb[:, u0:u0 + sz],
                                     A[:, fs], start=True, stop=True)
                stg = spool.tile([GMAX, b], F32)
                nc.scalar.activation(stg[0:sz, :], pp[0:sz, :], AF.Tanh)
                for f in range(0, b, MMF):
                    fs = slice(f, f + MMF)
                    nc.tensor.matmul(pbig[:, fs],
                                     w2g_sb[0:sz, (d - 1) * 2 * D:d * 2 * D],
                                     stg[0:sz, fs],
                                     start=(d == 1), stop=(d == D - 1),
                                     skip_group_check=True)
                els = epool.tile([D, b], F32)
                nc.scalar.activation(els[:], pbig[D:2 * D, :], AF.Exp,
                                     bias=b2x_sb[0:D, 1:2])
                tm = epool.tile([D, b], F32)
                nc.vector.tensor_tensor(tm[:], X[:], els[:], ALU.mult)
                yn = epool.tile([D, b], F32)
                nc.vector.scalar_tensor_tensor(yn[:], tm[:],
                                               b2x_sb[0:D, 0:1],
                                               pbig[0:D, :],
                                               ALU.add, ALU.add)
                nc.sync.dma_start(A[d:d + 1, :], yn[d:d + 1, :])
            nc.vector.tensor_scalar(lsout[:], pbig[D:2 * D, :],
                                    b2x_sb[0:D, 1:2], None, ALU.add)
            nc.sync.dma_start(yt[:, sl], A[0:D, :])
            nc.sync.dma_start(lt[:, sl], lsout[:])
    nc.compile()
    return nc


def _build_tri2(ns: int, b: int, chain: bool = True, use_f32r: bool = False,
                wv: int = 4):
    """Triangular kernel v4: dim-major waves of `wv` blocks.

    Each block owns ONE [128, b] PSUM tile (2 banks at b=1024): rows
    0:64 hold the accumulated params [shift|ls], rows 64:98 are the
    per-dim pre-activation scratch (matmul out at partition 64 -- legal
    for plain f32). DVE rewrites A[0:32] full-width each dim (rows <= d
    exactly final, rows > d finite garbage zeroed by mask1), so there
    is no per-dim DMA.
    """
    F32R = mybir.dt.float32r if use_f32r else F32
    _, sizes, offs = _group_info()
    nc = bacc.Bacc("TRN2", target_bir_lowering=False, debug=False,
                   num_devices=NCORES)
    assert ns % (wv * b) == 0
    xt2 = nc.dram_tensor("xt2", [2 * D, ns], F32, kind="ExternalInput")
    a0 = nc.dram_tensor("a0", [KA, ns], F32R, kind="ExternalInput")
    wa = nc.dram_tensor("wa", [KA, H], F32R, kind="ExternalInput")
    w2b = nc.dram_tensor("w2b", [P, (D - 1) * 2 * D], F32R,
                         kind="ExternalInput")
    b2v = nc.dram_tensor("b2v", [P, 2], F32, kind="ExternalInput")
    b2x = nc.dram_tensor("b2x", [D, 3], F32, kind="ExternalInput")
    yt = nc.dram_tensor("yt", [D, ns], F32, kind="ExternalOutput")
    lt = nc.dram_tensor("lt", [D, ns], F32, kind="ExternalOutput")

    nwave = ns // (wv * b)
    with tile.TileContext(nc) as tc, ExitStack() as ctx:
        cpool = ctx.enter_context(tc.tile_pool(name="const", bufs=1))
        apool = ctx.enter_context(tc.tile_pool(name="act", bufs=2))
        xpool = ctx.enter_context(tc.tile_pool(name="xf", bufs=1))
        spool = ctx.enter_context(tc.tile_pool(name="stg", bufs=2))
        epool = ctx.enter_context(tc.tile_pool(name="epi", bufs=2))
        opool = ctx.enter_context(tc.tile_pool(name="outs", bufs=1))
        prpool = ctx.enter_context(
            tc.tile_pool(name="pt", bufs=1, space=bass.MemorySpace.PSUM))

        wa_sb = cpool.tile([KA, H], F32R)
        w2b_sb = cpool.tile([P, (D - 1) * 2 * D], F32R)
        b2v_sb = cpool.tile([P, 2], F32)
        b2x_sb = cpool.tile([D, 3], F32)
        nc.sync.dma_start(wa_sb[:], wa[:])
        nc.sync.dma_start(w2b_sb[:], w2b[:])
        nc.sync.dma_start(b2v_sb[:], b2v[:])
        nc.sync.dma_start(b2x_sb[:], b2x[:])

        for w in range(nwave):
            blks = [wv * w + i for i in range(wv)]
            A = [apool.tile([KA, b], F32R, tag=f"A{i}", name=f"A{w}_{i}")
                 for i in range(wv)]
            Xf = [xpool.tile([2 * D, b], F32, tag=f"X{i}", name=f"Xf{w}_{i}")
                  for i in range(wv)]
            pt = [prpool.tile([P, b], F32, tag=f"pt{i}", name=f"pt{w}_{i}")
                  for i in range(wv)]
            for i in range(wv):
                nc.sync.dma_start(A[i][:, :],
                                  a0[:, blks[i] * b:(blks[i] + 1) * b])
                nc.sync.dma_start(Xf[i][:, :],
                                  xt2[:, blks[i] * b:(blks[i] + 1) * b])

            for d in range(1, D):
                sz, u0 = sizes[d - 1], int(offs[d - 1])
                gsl = slice((d - 1) * 2 * D, d * 2 * D)
                for i in range(wv):
                    for f in range(0, b, MMF):
                        fs = slice(f, f + MMF)
                        nc.tensor.matmul(pt[i][64:64 + sz, fs],
                                         wa_sb[:, u0:u0 + sz],
                                         A[i][:, fs], start=True, stop=True)
                sg = [spool.tile([GMAX, b], F32R, tag=f"sg{i}",
                                 name=f"sg{w}_{d}_{i}") for i in range(wv)]
                for i in range(wv):
                    nc.scalar.activation(sg[i][0:sz, :], pt[i][64:64 + sz, :],
                                         AF.Tanh)
                for i in range(wv):
                    for f in range(0, b, MMF):
                        fs = slice(f, f + MMF)
                        nc.tensor.matmul(pt[i][0:2 * D, fs],
                                         w2b_sb[0:sz, gsl],
                                         sg[i][0:sz, fs],
                                         start=(d == 1), stop=(d == D - 1),
                                         skip_group_check=True)
                els = [epool.tile([2 * D, b], F32, tag=f"els{i}",
                                  name=f"els{w}_{d}_{i}") for i in range(wv)]
                tm = [epool.tile([2 * D, b], F32, tag=f"tm{i}",
                                 name=f"tm{w}_{d}_{i}") for i in range(wv)]
                for i in range(wv):
                    nc.scalar.activation(els[i][:], pt[i][0:2 * D, :], AF.Exp,
                                         bias=b2v_sb[0:2 * D, 0:1])
                for i in range(wv):
                    nc.vector.tensor_tensor(tm[i][:], Xf[i][:], els[i][:],
                                            ALU.mult)
                for i in range(wv):
                    out = A[i][0:D, :] if chain else \
                        epool.tile([D, b], F32, tag=f"sc{i}",
                                   name=f"sc{w}_{d}_{i}")[:]
                    nc.vector.scalar_tensor_tensor(out,
                                                   tm[i][D:2 * D, :],
                                                   b2v_sb[D:2 * D, 1:2],
                                                   pt[i][0:D, :],
                                                   ALU.add, ALU.add)
            for i in range(wv):
                lso = opool.tile([D, b], F32, tag=f"ls{i}", name=f"ls{w}_{i}")
                yout = opool.tile([D, b], F32, tag=f"yo{i}", name=f"yo{w}_{i}")
                nc.vector.tensor_scalar(lso[:], pt[i][D:2 * D, :],
                                        b2x_sb[0:D, 1:2], None, ALU.add)
                nc.vector.tensor_copy(yout[:], A[i][0:D, :])
                nc.sync.dma_start(lt[:, blks[i] * b:(blks[i] + 1) * b],
                                  lso[:])
                nc.sync.dma_start(yt[:, blks[i] * b:(blks[i] + 1) * b],
                                  yout[:])
    nc.compile()
    return nc


def _pack_weights(W1, b1, Wc, W2, b2):
    mask1, mask2 = _masks()
    W1m = (np.asarray(W1) * mask1).astype(np.float32)
    W2m = (np.asarray(W2) * mask2[:, :, None]).astype(np.float32)
    Wa = np.concatenate([W1m, np.asarray(Wc, np.float32),
                         np.asarray(b1, np.float32)[None, :]], 0)   # [97,H]
    W2p = np.concatenate([W2m[:, :, 0], W2m[:, :, 1]], 1)           # [H,64]
    w2pack = np.ascontiguousarray(
        W2p.reshape(HCH, P, 2 * D).transpose(1, 0, 2).reshape(P, HCH * 2 * D))
    b2p = np.concatenate([np.asarray(b2, np.float32)[:, 0],
                          np.asarray(b2, np.float32)[:, 1]])[None, :]  # [1,64]
    return np.ascontiguousarray(Wa), w2pack, b2p


def _pack_weights_tri(W1, b1, Wc, W2, b2):
    perm, sizes, offs = _group_info()
    Wa, _, _ = _pack_weights(W1, b1, Wc, W2, b2)
    Was = np.ascontiguousarray(Wa[:, perm])
    mask1, mask2 = _masks()
    W2m = (np.asarray(W2) * mask2[:, :, None]).astype(np.float32)
    W2p = np.concatenate([W2m[:, :, 0], W2m[:, :, 1]], 1)[perm]   # sorted
    w2g = np.zeros((GMAX, (D - 1) * 2 * D), np.float32)
    for g in range(1, D):
        sz, u0 = sizes[g - 1], int(offs[g - 1])
        w2g[0:sz, (g - 1) * 2 * D:g * 2 * D] = W2p[u0:u0 + sz, :]
    b2 = np.asarray(b2, np.float32)
    b2x = np.zeros((D, 3), np.float32)
    b2x[:, 0] = b2[:, 0]
    b2x[:, 1] = b2[:, 1]
    b2x[0, 2] = np.exp(b2[0, 1])
    return Was, w2g, b2x


def _pack_weights_tri2(W1, b1, Wc, W2, b2, ns):
    perm, sizes, offs = _group_info()
    Was, w2g, b2x = _pack_weights_tri(W1, b1, Wc, W2, b2)
    mask1, mask2 = _masks()
    W2m = (np.asarray(W2) * mask2[:, :, None]).astype(np.float32)
    w2b = np.zeros((P, (D - 1) * 2 * D), np.float32)
    w2b[0:GMAX, :] = w2g
    b2 = np.asarray(b2, np.float32)
    b2v = np.zeros((P, 2), np.float32)
    b2v[D:2 * D, 0] = b2[:, 1]
    b2v[2 * D:3 * D, 0] = b2[:, 1]
    b2v[D:2 * D, 1] = b2[:, 0]
    b2v[2 * D:3 * D, 1] = b2[:, 0]
    return Was, w2b, b2v, b2x


def _pack_x2(x_shard_T):
    """xt2 [64, ns]: rows 32:64 = x dims, rest zero."""
    ns = x_shard_T.shape[1]
    xt2 = np.zeros((2 * D, ns), np.float32)
    xt2[D:2 * D, :] = x_shard_T
    return xt2


_CACHE = {}


def _get_nc(key, builder):
    if key not in _CACHE:
        _CACHE[key] = builder()
    return _CACHE[key]


B_TRI = 1024


def kernel(x, context, W1, b1, Wc, W2, b2):
    x = np.asarray(x, np.float32)
    context = np.asarray(context, np.float32)
    b2 = np.asarray(b2, np.float32)
    Was, w2b, b2v, b2x = _pack_weights_tri2(W1, b1, Wc, W2, b2, NS)

    nc = _get_nc(("tri3", NS, B_TRI), lambda: _build_tri2(NS, B_TRI))

    in_maps = []
    for c in range(NCORES):
        rows = slice(c * NS, (c + 1) * NS)
        a0 = np.empty((KA, NS), np.float32)
        a0[0:D] = 0.0
        a0[0] = x[rows, 0] * np.exp(b2[0, 1]) + b2[0, 0]
        a0[D:D + C] = context[rows].T
        a0[D + C:] = 1.0
        in_maps.append({
            "xt2": _pack_x2(np.ascontiguousarray(x[rows].T)),
            "a0": a0,
            "wa": Was, "w2b": w2b, "b2v": b2v, "b2x": b2x,
        })
    res = bass_utils.run_bass_kernel_spmd(nc, in_maps,
                                          core_ids=list(range(NCORES)))
    global LAST_RESULT, LAST_IN_MAPS
    LAST_RESULT = res
    LAST_IN_MAPS = in_maps
    y = np.empty((N, D), np.float32)
    ld = np.empty((N,), np.float32)
    for c, out in enumerate(res.results):
        rows = slice(c * NS, (c + 1) * NS)
        y[rows] = out["yt"].T
        ld[rows] = out["lt"].sum(axis=0)
    return y, ld
